# revision 10
# baseline (speedup 1.0000x reference)
"""BitLinear (ternary-quantized linear) TRN2 Bass kernel, 8-way tensor-parallel.

Reference semantics (fp32):
    gamma = mean(|W|)                      # W: [D_OUT, D_IN]
    w_q   = clip(round(W / gamma), -1, 1)  # ternary {-1, 0, 1}
    out   = gamma * (x @ w_q^T)            # x: [B, S, D_IN]

Sharding: W rows (out_features) split across 8 cores; x replicated. gamma
needs the global |W| sum -> tiny scalar AllReduce across the 8 cores.

Per-core pipeline:
  1. load W shard [512, 4096] (natural layout), abs-sum -> partial scalar
  2. AllReduce partial sums -> gamma, thresholds g2 = gamma/2
  3. load W shard transposed [k, feat] tiles, quantize:
       w_q = (w >= g2) - (w <= -g2)   (equivalent to clip(round(w/gamma)))
  4. stream x^T tiles [k=128, tok], matmul-accumulate over k into PSUM
     (dtype float32r: 1 cycle/row on the PE vs 4 for fp32; ~1e-4 rel err),
     scale by gamma on PSUM eviction, DMA out.

Output assembled host-side by concatenating the 8 feature shards.
"""

import os
import sys

sys.path.insert(0, "/opt/trn_rl_repo")

import numpy as np

import concourse.bass as bass
import concourse.tile as tile
from concourse import bacc, mybir

dt = mybir.dt

B, S, D_IN, D_OUT = 4, 2048, 4096, 4096
N_CORES = 8

# "f32r": single-pass float32r matmul (fast; ~1e-4 rel-to-absmax err)
# "x2":   bf16 hi/lo split on x, 2 matmuls (2x PE time; ~3e-6 err)
MODE = os.environ.get("BITLINEAR_MODE", "f32r")


def build(b=None, s=None, d_in=None, d_out=None, n_cores=None, mode=None):
    """Trace + compile the per-core SPMD program. Returns the Bacc module."""
    b = B if b is None else b
    s = S if s is None else s
    d_in = D_IN if d_in is None else d_in
    d_out = D_OUT if d_out is None else d_out
    n_cores = N_CORES if n_cores is None else n_cores
    mode = MODE if mode is None else mode
    toks = b * s
    o_shard = d_out // n_cores  # 512: features per core
    KT = d_in // 128  # 32 k-tiles
    CHUNK = 1024 if toks % 1024 == 0 else 128  # tokens per chunk (8 psum banks)
    CHUNK = int(os.environ.get("BITLINEAR_CHUNK", CHUNK))
    TB = CHUNK // 128  # token blocks per chunk
    n_chunks = toks // CHUNK
    WT = d_in  # free size of natural W tiles
    FP = o_shard // 128  # feature-partition tiles of W shard (4)
    n_elem = float(d_in * d_out)

    nc = bacc.Bacc(
        "TRN2",
        target_bir_lowering=False,
        debug=False,
        enable_asserts=False,
        num_devices=n_cores,
    )

    x_d = nc.dram_tensor("x", [toks, d_in], dt.float32, kind="ExternalInput").ap()
    w_d = nc.dram_tensor("w", [o_shard, d_in], dt.float32, kind="ExternalInput").ap()
    out_d = nc.dram_tensor(
        "out", [toks, o_shard], dt.float32, kind="ExternalOutput"
    ).ap()

    cc_in = nc.dram_tensor("cc_in", [128], dt.float32)
    cc_out = nc.dram_tensor("cc_out", [128], dt.float32, addr_space="Shared")

    mm_dt = dt.float32r if mode == "f32r" else dt.bfloat16

    with tile.TileContext(nc) as tc:
        with (
            tc.tile_pool(name="const", bufs=1) as const,
            tc.tile_pool(name="gphase", bufs=2) as gphase,
            tc.tile_pool(name="wq", bufs=1) as wqp,
            tc.tile_pool(name="quant", bufs=3) as quant,
            tc.tile_pool(name="xin", bufs=4) as xin,
            tc.tile_pool(name="xr", bufs=4) as xrp,
            tc.tile_pool(name="evac", bufs=2 * TB) as evac,
            tc.tile_pool(name="ps", bufs=1, space="PSUM") as psp,
        ):
            # ---- Phase G: partial |W| sum ------------------------------------
            ones = const.tile([128, 1], dt.float32)
            nc.vector.memset(ones[:], 1.0)
            asum = const.tile([128, FP], dt.float32)
            for fp in range(FP):
                wt = gphase.tile([128, WT], dt.float32, tag="wnat")
                nc.sync.dma_start(out=wt[:], in_=w_d[fp * 128 : (fp + 1) * 128, :])
                st = gphase.tile([128, WT // 128], dt.float32, tag="stage")
                nc.vector.tensor_reduce(
                    st[:],
                    wt[:].rearrange("p (a c) -> p a c", c=128),
                    axis=mybir.AxisListType.X,
                    op=mybir.AluOpType.add,
                    apply_absolute_value=True,
                )
                nc.vector.reduce_sum(
                    asum[:, fp : fp + 1], st[:], axis=mybir.AxisListType.X
                )
            asum1 = const.tile([128, 1], dt.float32)
            nc.vector.reduce_sum(asum1[:], asum[:], axis=mybir.AxisListType.X)
            # partition sum via PE: asum1.T @ ones -> [1, 1]
            psum_t = psp.tile([1, 1], dt.float32, tag="ps0", name="gsum_ps")
            nc.tensor.matmul(psum_t[:], asum1[:], ones[:], start=True, stop=True)
            part = const.tile([1, 1], dt.float32)
            nc.scalar.copy(part[:], psum_t[:])

            # ---- Phase A: AllReduce partial sums -----------------------------
            if n_cores > 1:
                # pad the collective payload to 512 B; only lane 0 is used
                ccz = const.tile([1, 128], dt.float32)
                nc.vector.memset(ccz[:], 0.0)
                nc.scalar.copy(ccz[:1, :1], part[:1, :1])
                nc.sync.dma_start(out=cc_in[:], in_=ccz[0, :])
                nc.gpsimd.collective_compute(
                    "AllReduce",
                    mybir.AluOpType.add,
                    ins=[cc_in[:]],
                    outs=[cc_out[:]],
                    replica_groups=[list(range(n_cores))],
                )
                tsum_src = bass.AP(tensor=cc_out, offset=0, ap=[[0, 128], [1, 1]])
            else:
                tsum_src = None
            tsum = const.tile([128, 1], dt.float32)
            if tsum_src is not None:
                nc.sync.dma_start(out=tsum[:], in_=tsum_src)
            else:
                nc.vector.tensor_copy(tsum[:], part[:].broadcast(0, 128))
            g2 = const.tile([128, 1], dt.float32)
            ng2 = const.tile([128, 1], dt.float32)
            gam = const.tile([128, 1], dt.float32)
            nc.scalar.mul(g2[:], tsum[:], 0.5 / n_elem)
            nc.scalar.mul(ng2[:], tsum[:], -0.5 / n_elem)
            nc.scalar.mul(gam[:], tsum[:], 1.0 / n_elem)

            # ---- Phase Q: quantize W^T tiles ---------------------------------
            # w^T tile [k=128, feat] loaded with transposed access from w_d.
            wq = wqp.tile([128, KT, o_shard], mm_dt)
            for kt in range(KT):
                wtt = quant.tile([128, o_shard], dt.float32, tag="wtt")
                nc.sync.dma_start(
                    out=wtt[:],
                    in_=w_d[:, kt * 128 : (kt + 1) * 128].rearrange("f k -> k f"),
                )
                pos = quant.tile([128, o_shard], dt.float32, tag="pos")
                neg = quant.tile([128, o_shard], dt.float32, tag="neg")
                nc.vector.tensor_scalar(
                    pos[:], wtt[:], g2[:], None, op0=mybir.AluOpType.is_ge
                )
                nc.vector.tensor_scalar(
                    neg[:], wtt[:], ng2[:], None, op0=mybir.AluOpType.is_le
                )
                nc.vector.tensor_tensor(
                    wq[:, kt, :], pos[:], neg[:], op=mybir.AluOpType.subtract
                )

            # ---- Phase M: matmul over token chunks ---------------------------
            for ch in range(n_chunks):
                t0 = ch * CHUNK
                pss = [
                    psp.tile(
                        [128, o_shard], dt.float32, tag=f"ps{tb}", name=f"ps{tb}_{ch}"
                    )
                    for tb in range(TB)
                ]
                for kt in range(KT):
                    xt = xin.tile([128, CHUNK], dt.float32, tag="xt")
                    nc.sync.dma_start(
                        out=xt[:],
                        in_=x_d[t0 : t0 + CHUNK, kt * 128 : (kt + 1) * 128].rearrange(
                            "t k -> k t"
                        ),
                    )
                    if mode == "f32r":
                        xr = xrp.tile([128, CHUNK], dt.float32r, tag="xr")
                        nc.vector.tensor_copy(xr[:], xt[:])
                        lhs_list = [xr]
                    else:
                        xhi = xrp.tile([128, CHUNK], dt.bfloat16, tag="xhi")
                        xhi32 = xrp.tile([128, CHUNK], dt.float32, tag="xhi32")
                        xlo = xrp.tile([128, CHUNK], dt.bfloat16, tag="xlo")
                        nc.vector.tensor_copy(xhi[:], xt[:])
                        nc.scalar.copy(xhi32[:], xhi[:])
                        nc.vector.tensor_tensor(
                            xlo[:], xt[:], xhi32[:], op=mybir.AluOpType.subtract
                        )
                        lhs_list = [xhi, xlo]
                    n_acc = len(lhs_list) * KT
                    for tb in range(TB):
                        for li, lhs in enumerate(lhs_list):
                            i_acc = kt * len(lhs_list) + li
                            nc.tensor.matmul(
                                pss[tb][:],
                                lhs[:, tb * 128 : (tb + 1) * 128],
                                wq[:, kt, :],
                                start=(i_acc == 0),
                                stop=(i_acc == n_acc - 1),
                            )
                for tb in range(TB):
                    ot = evac.tile([128, o_shard], dt.float32, tag="ot")
                    nc.scalar.activation(
                        ot[:],
                        pss[tb][:],
                        mybir.ActivationFunctionType.Copy,
                        scale=gam[:],
                    )
                    nc.sync.dma_start(
                        out=out_d[t0 + tb * 128 : t0 + (tb + 1) * 128, :], in_=ot[:]
                    )

    nc.compile()
    return nc


# ---------------------------------------------------------------------------
# Execution: cached jitted SPMD callable (modeled on bass2jax.run_bass_via_pjrt,
# but reusable across calls so repeat timing excludes host->device upload).
# ---------------------------------------------------------------------------
_CACHE = {}


def _get_runner():
    if "runner" in _CACHE:
        return _CACHE["runner"]

    import jax
    from jax.sharding import Mesh, PartitionSpec
    from jax.experimental.shard_map import shard_map
    from concourse import bass2jax
    from concourse.bass2jax import (
        _bass_exec_p,
        install_neuronx_cc_hook,
        partition_id_tensor,
    )

    install_neuronx_cc_hook()
    nc = build()
    partition_name = nc.partition_id_tensor.name if nc.partition_id_tensor else None

    in_names, out_names, out_avals = [], [], []
    for alloc in nc.m.functions[0].allocations:
        if not isinstance(alloc, mybir.MemoryLocationSet):
            continue
        name = alloc.memorylocations[0].name
        if alloc.kind == "ExternalInput":
            if name != partition_name:
                in_names.append(name)
        elif alloc.kind == "ExternalOutput":
            out_names.append(name)
            out_avals.append(
                jax.core.ShapedArray(tuple(alloc.tensor_shape), mybir.dt.np(alloc.dtype))
            )
    n_params = len(in_names)
    all_in_names = list(in_names) + out_names
    if partition_name is not None:
        all_in_names.append(partition_name)

    def _body(*args):
        operands = list(args)
        if partition_name is not None:
            operands.append(partition_id_tensor())
        outs = _bass_exec_p.bind(
            *operands,
            out_avals=tuple(out_avals),
            in_names=tuple(all_in_names),
            out_names=tuple(out_names),
            lowering_input_output_aliases=(),
            sim_require_finite=True,
            sim_require_nnan=True,
            nc=nc,
        )
        return tuple(outs)

    devices = jax.devices()[:N_CORES]
    mesh = Mesh(np.asarray(devices), ("core",))
    n_args = n_params + len(out_names)
    sharded = jax.jit(
        shard_map(
            _body,
            mesh=mesh,
            in_specs=(PartitionSpec("core"),) * n_args,
            out_specs=(PartitionSpec("core"),) * len(out_names),
            check_rep=False,
        ),
        keep_unused=True,
    )
    _CACHE["runner"] = (sharded, in_names, out_names, out_avals, mesh)
    return _CACHE["runner"]


def _device_inputs(x2d, weight):
    """Concatenated per-core input arrays (axis 0), order matching in_names."""
    sharded, in_names, out_names, out_avals, mesh = _get_runner()
    o_shard = D_OUT // N_CORES
    per_core = {
        "x": [x2d] * N_CORES,
        "w": [weight[c * o_shard : (c + 1) * o_shard] for c in range(N_CORES)],
    }
    ins = [np.concatenate(per_core[n], axis=0) for n in in_names]
    zouts = [
        np.zeros((N_CORES * a.shape[0],) + a.shape[1:], a.dtype) for a in out_avals
    ]
    return ins + zouts


def kernel(x: np.ndarray, weight: np.ndarray) -> np.ndarray:
    assert x.shape == (B, S, D_IN) and weight.shape == (D_OUT, D_IN)
    x2d = np.ascontiguousarray(np.asarray(x, dtype=np.float32).reshape(B * S, D_IN))
    weight = np.ascontiguousarray(np.asarray(weight, dtype=np.float32))

    sharded, in_names, out_names, out_avals, mesh = _get_runner()
    args = _device_inputs(x2d, weight)
    out_arrs = sharded(*args)
    out_idx = out_names.index("out")
    full = np.asarray(out_arrs[out_idx])  # [N_CORES * toks, o_shard]
    toks = B * S
    o_shard = D_OUT // N_CORES
    shards = full.reshape(N_CORES, toks, o_shard)
    out2d = np.concatenate(list(shards), axis=1)  # [toks, D_OUT]
    return out2d.reshape(B, S, D_OUT).astype(np.float32)
